# revision 8
# baseline (speedup 1.0000x reference)
"""Trainium2 Bass kernel for nn_CrossAttention (B=8, L=1024, QD=1024, KVD=768, H=16).

Sharding: data-parallel over batch across the 8 NeuronCores (1 batch row each).
Per-core pipeline (bf16 matmuls, fp32 accumulation / residual / layernorm):
  A) per-128-row-block plain fp32 loads (HWDGE, full DMA rate), fp32->bf16
     conversion on compute engines (DVE for q/wq/k/wk, GpSimd for v/wv/wo —
     SWDGE converting DMA is ~5x slower than line rate, so it is avoided),
     then SBUF->SBUF xbar DMA transposes into block-major transposed layouts
     XTb[p, rb, cc, r] = X[rb*128+r, cc*128+p] (contiguous dst = fast path).
  B) projections: qhT/khT (per-partition bias; B1 evictions on DVE, B2 on
     ACT), vh natural with ones-augmented columns; key-padding mask folded
     into vh_aug rows so exp needs no mask bias. B3 interleaved into the
     first attention pair slots.
  C) attention per head pair: scoresT = khT.T @ qhT with the two heads of a
     pair issued back-to-back on distinct PE row groups (64-row tiling =>
     concurrent execution); exp split across engines: hh=0 on ACT (table
     exp), hh=1 on DVE via a one-instruction Schraudolph bit-trick straight
     to bf16 (the constant-factor part of its error cancels in softmax);
     attnV with [ones|vh] stationary giving psum rows 0:64 = replicated
     denominator and rows 64:128 = o; approx reciprocal + multiply on DVE.
  D) out-projection from oT stationary + rank-1 bias, fp32 residual + LN.
"""

import numpy as np

import concourse.bass as bass
import concourse.mybir as mybir
import concourse.tile as tile
from concourse import bacc
from concourse.bass_utils import run_bass_kernel_spmd

F32 = mybir.dt.float32
BF16 = mybir.dt.bfloat16
I16 = mybir.dt.int16
U8 = mybir.dt.uint8

B = 8
L = 1024
C = 1024      # QD
KV = 768      # KVD
H = 16
DH = 64
P = 128
LT = L // P          # 8 l-tiles
CT = C // P          # 8 contraction tiles (model dim)
KT = KV // P         # 6 contraction tiles (kv dim)
DT = C // P          # 8 d-tiles
NH = C // 512        # 2 free-dim halves (N=512 per PSUM bank)
SCALE = DH ** -0.5
EPS = 1e-5
LOG2E = 1.4426950408889634
# Schraudolph exp straight to bf16 bits: u16 = x*EXP_A + EXP_B, bitcast bf16.
EXP_A = 128.0 * LOG2E * SCALE
EXP_B = 128.0 * (127.0 - 0.043677) + 0.5

Exp = mybir.ActivationFunctionType.Exp
Sqrt = mybir.ActivationFunctionType.Sqrt
Identity = mybir.ActivationFunctionType.Identity
MULT = mybir.AluOpType.mult
ADD = mybir.AluOpType.add

TRACE = False
LAST_RESULT = None
_CACHE = {}


def _bcast_ap(handle, parts):
    apx = handle[:]
    return bass.AP(tensor=apx.tensor, offset=apx.offset,
                   ap=[[0, parts]] + [list(x) for x in apx.ap])


def build(apply_gb=False):
    nc = bacc.Bacc("TRN2", target_bir_lowering=False)

    q_in = nc.dram_tensor("q", [L, C], F32, kind="ExternalInput")
    k_in = nc.dram_tensor("k", [L, KV], F32, kind="ExternalInput")
    v_in = nc.dram_tensor("v", [L, KV], F32, kind="ExternalInput")
    m_in = nc.dram_tensor("key_padding_mask", [L], U8, kind="ExternalInput")
    wq_in = nc.dram_tensor("Wq", [C, C], F32, kind="ExternalInput")
    bq_in = nc.dram_tensor("bq", [C], F32, kind="ExternalInput")
    wk_in = nc.dram_tensor("Wk", [C, KV], F32, kind="ExternalInput")
    bk_in = nc.dram_tensor("bk", [C], F32, kind="ExternalInput")
    wv_in = nc.dram_tensor("Wv", [C, KV], F32, kind="ExternalInput")
    bv_in = nc.dram_tensor("bv", [C], F32, kind="ExternalInput")
    wo_in = nc.dram_tensor("Wo", [C, C], F32, kind="ExternalInput")
    bo_in = nc.dram_tensor("bo", [C], F32, kind="ExternalInput")
    gamma_in = nc.dram_tensor("gamma", [C], F32, kind="ExternalInput")
    beta_in = nc.dram_tensor("beta", [C], F32, kind="ExternalInput")
    y_out = nc.dram_tensor("y", [L, C], F32, kind="ExternalOutput")

    with tile.TileContext(nc) as tc:
        with (
            tc.tile_pool(name="cst", bufs=1) as cst,
            tc.tile_pool(name="persist", bufs=1) as persist,
            tc.tile_pool(name="xpO", bufs=1) as xpO,
        ):
            # ---------------- projection outputs (persist through attention)
            qhT = persist.tile([P, DT, L], BF16)          # d on partitions
            khT = persist.tile([P, DT, L], BF16)
            vh_aug = persist.tile([P, LT, H * P], BF16)   # per m-tile: 16x[64 ones | 64 vh]
            WoTb = xpO.tile([P, CT, DT, P], BF16)         # [p, cb, dc, r]
            oT = xpO.tile([P, DT, L], BF16)

            with (
                tc.tile_pool(name="xpV", bufs=1) as xpV,
                tc.tile_pool(name="stg32", bufs=3) as stg32,
                tc.tile_pool(name="stg", bufs=4) as stg,
            ):
                WvTb = xpV.tile([P, DT, KT, P], BF16)
                vTb = xpV.tile([P, LT, KT, P], BF16)

                def stage_block(nm, hnd, cols, b, dstT, conv):
                    # plain fp32 block load at full DMA rate, engine-side
                    # cast to bf16, then SBUF->SBUF xbar transpose into the
                    # block-major transposed world (contiguous destination):
                    # dstT[p, b, cc, r] = X[b*128+r, cc*128+p].
                    sf = stg32.tile([P, cols], F32, name=f"sf_{nm}{b}",
                                    tag="stg32")
                    nc.scalar.dma_start(sf, hnd[b * P:(b + 1) * P, :])
                    st = stg.tile([P, cols], BF16, name=f"st_{nm}{b}",
                                  tag="stg")
                    conv.tensor_copy(st, sf)
                    nc.sync.dma_start(dstT[:, b, :, :], st, transpose=True)

                with (
                    tc.tile_pool(name="xpQK", bufs=1) as xpQK,
                    tc.tile_pool(name="psum_b", bufs=3, space="PSUM") as psum_b,
                ):
                    WqTb = xpQK.tile([P, DT, CT, P], BF16)
                    qTb = xpQK.tile([P, LT, CT, P], BF16)
                    WkTb = xpQK.tile([P, DT, KT, P], BF16)
                    kTb = xpQK.tile([P, LT, KT, P], BF16)

                    # tiny consts first (they gate vh masking / evictions)
                    bq_sb = cst.tile([P, DT], F32)
                    nc.gpsimd.dma_start(bq_sb, bq_in[:].rearrange("(t p) -> p t", p=P))
                    bk_sb = cst.tile([P, DT], F32)
                    nc.gpsimd.dma_start(bk_sb, bk_in[:].rearrange("(t p) -> p t", p=P))
                    mask_u8 = cst.tile([P, LT], U8)
                    nc.gpsimd.dma_start(mask_u8, m_in[:].rearrange("(t p) -> p t", p=P))
                    mask01 = cst.tile([P, LT], F32)
                    nc.vector.tensor_copy(mask01, mask_u8)
                    ones_row = cst.tile([1, P], BF16)
                    nc.vector.memset(ones_row, 1.0)
                    eps_sb = cst.tile([P, 1], F32)
                    nc.vector.memset(eps_sb, EPS)
                    bv_bf = cst.tile([1, C], BF16)
                    nc.gpsimd.dma_start(bv_bf, bv_in[:].rearrange("(a c) -> a c", a=1))
                    bo_bf = cst.tile([1, C], BF16)
                    nc.gpsimd.dma_start(bo_bf, bo_in[:].rearrange("(a c) -> a c", a=1))
                    if apply_gb:
                        gamma_b = cst.tile([P, C], F32)
                        nc.gpsimd.dma_start(gamma_b, _bcast_ap(gamma_in, P))
                        beta_b = cst.tile([P, C], F32)
                        nc.gpsimd.dma_start(beta_b, _bcast_ap(beta_in, P))
                    else:
                        gamma_b = beta_b = None

                    # vh_aug: ones columns masked by key_padding (folds the
                    # mask out of exp entirely: padded m-rows contribute 0 to
                    # both numerator and denominator).
                    nc.gpsimd.memset(vh_aug[:], 1.0)
                    for mt in range(LT):
                        ones_cols = vh_aug[:, mt, :].rearrange(
                            "p (h x) -> p h x", x=P)[:, :, 0:DH]
                        nc.vector.tensor_scalar_mul(ones_cols, ones_cols,
                                                    mask01[:, mt:mt + 1])

                    # ---- stage q/wq (gates B1), then k/wk (B2 runs behind)
                    for b in range(4):
                        stage_block("q", q_in, C, b, qTb, nc.vector)
                    stage_block("wq", wq_in, C, 0, WqTb, nc.vector)
                    for b in range(4, 8):
                        stage_block("q", q_in, C, b, qTb, nc.vector)
                    for b in range(1, 8):
                        stage_block("wq", wq_in, C, b, WqTb, nc.vector)
                    for b in range(4):
                        stage_block("k", k_in, KV, b, kTb, nc.vector)
                    stage_block("wk", wk_in, KV, 0, WkTb, nc.vector)
                    for b in range(4, 8):
                        stage_block("k", k_in, KV, b, kTb, nc.vector)
                    for b in range(1, 8):
                        stage_block("wk", wk_in, KV, b, WkTb, nc.vector)
                    # v/wv/wo staged behind on the same DMA queues; their
                    # casts run on GpSimd (1-input ops are ~line rate there)
                    # keeping DVE/ACT free for the attention era.
                    stage_block("v", v_in, KV, 0, vTb, nc.gpsimd)
                    for b in range(4):
                        stage_block("wv", wv_in, KV, b, WvTb, nc.gpsimd)
                    stage_block("v", v_in, KV, 1, vTb, nc.gpsimd)
                    for b in range(4, 8):
                        stage_block("wv", wv_in, KV, b, WvTb, nc.gpsimd)
                    for b in range(2, 8):
                        stage_block("v", v_in, KV, b, vTb, nc.gpsimd)
                    for b in range(8):
                        stage_block("wo", wo_in, C, b, WoTb, nc.gpsimd)

                    # ---- B1: qhT[d, l]   (evictions on DVE)
                    for dt in range(DT):
                        for lh in range(NH):
                            ps = psum_b.tile([P, 512], F32, tag="ps")
                            for ct in range(CT):
                                nc.tensor.matmul(
                                    ps, WqTb[:, dt, ct, :],
                                    qTb[:, lh * 4:(lh + 1) * 4, ct, :],
                                    start=(ct == 0), stop=(ct == CT - 1))
                            nc.vector.tensor_scalar_add(
                                qhT[:, dt, lh * 512:(lh + 1) * 512], ps,
                                bq_sb[:, dt:dt + 1])

                    # ---- B2: khT[d, l]   (evictions on ACT)
                    for dt in range(DT):
                        for lh in range(NH):
                            ps = psum_b.tile([P, 512], F32, tag="ps")
                            for ct in range(KT):
                                nc.tensor.matmul(
                                    ps, WkTb[:, dt, ct, :],
                                    kTb[:, lh * 4:(lh + 1) * 4, ct, :],
                                    start=(ct == 0), stop=(ct == KT - 1))
                            nc.scalar.activation(
                                khT[:, dt, lh * 512:(lh + 1) * 512], ps,
                                Identity, bias=bk_sb[:, dt:dt + 1])

                # ---------------- attention, with B3 (vh projection)
                # interleaved into the first two pair slots
                with (
                    tc.tile_pool(name="ptp", bufs=26) as ptp,
                    tc.tile_pool(name="recp", bufs=2) as recp,
                    tc.tile_pool(name="psum_sc", bufs=2, space="PSUM") as psum_sc,
                    tc.tile_pool(name="psum_av", bufs=1, space="PSUM") as psum_av,
                    tc.tile_pool(name="psum_b3", bufs=2, space="PSUM") as psum_b3,
                ):
                    pts = {}

                    def scores_exp(pair):
                        for mt in range(LT):
                            sc = []
                            for hh in range(2):
                                s = psum_sc.tile([P, L], F32,
                                                 name=f"sc{pair}_{mt}_{hh}", tag="sc")
                                sc.append(s)
                            # interleave the two heads' matmuls: distinct
                            # 64-row PE groups execute them concurrently
                            for lh in range(NH):
                                for hh in range(2):
                                    p0 = hh * DH
                                    nc.tensor.matmul(
                                        sc[hh][:, lh * 512:(lh + 1) * 512],
                                        khT[p0:p0 + DH, pair, mt * P:(mt + 1) * P],
                                        qhT[p0:p0 + DH, pair, lh * 512:(lh + 1) * 512],
                                        start=True, stop=True)
                            for hh in range(2):
                                pt = ptp.tile([P, L], BF16,
                                              name=f"pt{pair}_{mt}_{hh}", tag="pt")
                                pts[(pair, mt, hh)] = pt
                                if hh == 0:
                                    nc.scalar.activation(pt, sc[hh], Exp,
                                                         scale=SCALE)
                                else:
                                    nc.vector.tensor_scalar(
                                        pt[:].bitcast(I16), sc[hh],
                                        EXP_A, EXP_B, MULT, ADD)

                    def b3_chunk(mts):
                        for mt in mts:
                            for dh2 in range(NH):
                                ps = psum_b3.tile([P, 512], F32, tag="ps3")
                                for ct in range(KT):
                                    nc.tensor.matmul(
                                        ps, vTb[:, mt, ct, :],
                                        WvTb[:, dh2 * 4:(dh2 + 1) * 4, ct, :],
                                        start=(ct == 0), stop=False)
                                nc.tensor.matmul(
                                    ps, ones_row[0:1, :],
                                    bv_bf[0:1, dh2 * 512:(dh2 + 1) * 512],
                                    start=False, stop=True)
                                dst = vh_aug[:, mt, :].rearrange(
                                    "p (h x) -> p h x", x=P)
                                dst = dst[:, dh2 * 8:(dh2 + 1) * 8, DH:P]
                                # eviction also applies the key-padding mask
                                nc.vector.tensor_scalar_mul(
                                    dst, ps[:].rearrange("p (h d) -> p h d", d=DH),
                                    mask01[:, mt:mt + 1])

                    def attnv(pair):
                        for hh in range(2):
                            h = 2 * pair + hh
                            av = psum_av.tile([P, L], F32,
                                              name=f"av{pair}_{hh}", tag="av")
                            # mt-outer: both l-halves reuse each vh stationary load
                            for mt in range(LT):
                                for lh in range(NH):
                                    nc.tensor.matmul(
                                        av[:, lh * 512:(lh + 1) * 512],
                                        vh_aug[:, mt, h * P:(h + 1) * P],
                                        pts[(pair, mt, hh)][:, lh * 512:(lh + 1) * 512],
                                        start=(mt == 0), stop=(mt == LT - 1))
                            rec = recp.tile([P, L], F32,
                                            name=f"rec{pair}_{hh}", tag="rec")
                            nc.vector.reciprocal_approx_fast(rec[0:DH, :],
                                                             av[0:DH, :])
                            nc.vector.tensor_mul(
                                oT[hh * DH:(hh + 1) * DH, pair, :],
                                av[DH:P, :], rec[0:DH, :])
                            for mt in range(LT):
                                del pts[(pair, mt, hh)]

                    scores_exp(0)
                    b3_chunk(range(0, 4))
                    scores_exp(1)
                    b3_chunk(range(4, 8))
                    attnv(0)
                    for pair in range(2, H // 2):
                        scores_exp(pair)
                        attnv(pair - 1)
                    attnv(H // 2 - 1)

            # ---------------- out-projection + residual + layernorm
            with (
                tc.tile_pool(name="dwork", bufs=3) as dwork,
                tc.tile_pool(name="dsmall", bufs=8) as dsmall,
                tc.tile_pool(name="psum_y", bufs=3, space="PSUM") as psum_y,
            ):
                qrs = []
                for lt in range(LT):
                    qr = dwork.tile([P, C], F32, name=f"qr{lt}", tag="qr",
                                    bufs=8)
                    nc.sync.dma_start(qr, q_in[lt * P:(lt + 1) * P, :])
                    qrs.append(qr)
                for lt in range(LT):
                    yp = psum_y.tile([P, C], F32, tag="yp")
                    # dt-outer: the oT stationary serves both column halves
                    for dt in range(DT):
                        for ch in range(NH):
                            nc.tensor.matmul(
                                yp[:, ch * 512:(ch + 1) * 512],
                                oT[:, dt, lt * P:(lt + 1) * P],
                                WoTb[:, ch * 4:(ch + 1) * 4, dt, :],
                                start=(dt == 0), stop=False)
                    for ch in range(NH):
                        nc.tensor.matmul(
                            yp[:, ch * 512:(ch + 1) * 512],
                            ones_row[0:1, :],
                            bo_bf[0:1, ch * 512:(ch + 1) * 512],
                            start=False, stop=True)
                    ysb = dwork.tile([P, C], F32, tag="ysb")
                    nc.vector.tensor_add(ysb, yp, qrs[lt])
                    st = dsmall.tile([P, 2, 6], F32, tag="st")
                    nc.vector.bn_stats(st[:, 0, :], ysb[:, 0:512])
                    nc.vector.bn_stats(st[:, 1, :], ysb[:, 512:1024])
                    mv = dsmall.tile([P, 2], F32, tag="mv")
                    nc.vector.bn_aggr(mv, st)
                    rstd = dsmall.tile([P, 1], F32, tag="rstd")
                    nc.scalar.activation(rstd, mv[:, 1:2], Sqrt,
                                         bias=eps_sb[:, 0:1])
                    nc.vector.reciprocal(rstd, rstd)
                    nmr = dsmall.tile([P, 1], F32, tag="nmr")
                    nc.vector.tensor_mul(nmr, mv[:, 0:1], rstd)
                    nc.vector.tensor_scalar_mul(nmr, nmr, -1.0)
                    yn = dwork.tile([P, C], F32, tag="yn")
                    nc.scalar.activation(yn, ysb, Identity, bias=nmr[:, 0:1],
                                         scale=rstd[:, 0:1])
                    if apply_gb:
                        nc.vector.tensor_mul(yn, yn, gamma_b)
                        nc.gpsimd.tensor_add(yn, yn, beta_b)
                    nc.sync.dma_start(y_out[lt * P:(lt + 1) * P, :], yn)

    nc.compile()
    return nc


def _get_nc(apply_gb):
    key = ("nc", apply_gb)
    if key not in _CACHE:
        _CACHE[key] = build(apply_gb)
    return _CACHE[key]


def kernel(**inputs) -> np.ndarray:
    global LAST_RESULT
    gamma = np.asarray(inputs["gamma"], dtype=np.float32)
    beta = np.asarray(inputs["beta"], dtype=np.float32)
    apply_gb = not (np.all(gamma == 1.0) and np.all(beta == 0.0))
    nc = _get_nc(apply_gb)
    q = np.ascontiguousarray(np.asarray(inputs["q"], dtype=np.float32))
    k = np.ascontiguousarray(np.asarray(inputs["k"], dtype=np.float32))
    v = np.ascontiguousarray(np.asarray(inputs["v"], dtype=np.float32))
    mask = np.ascontiguousarray(np.asarray(inputs["key_padding_mask"]).astype(np.uint8))
    shared = {
        name: np.ascontiguousarray(np.asarray(inputs[name], dtype=np.float32))
        for name in ("Wq", "bq", "Wk", "bk", "Wv", "bv", "Wo", "bo", "gamma", "beta")
    }
    in_maps = []
    for b in range(B):
        m = {"q": q[b], "k": k[b], "v": v[b], "key_padding_mask": mask[b]}
        m.update(shared)
        in_maps.append(m)
    LAST_RESULT = run_bass_kernel_spmd(nc, in_maps, core_ids=list(range(B)), trace=TRACE)
    return np.stack([r["y"] for r in LAST_RESULT.results], axis=0)


# revision 11
# speedup vs baseline: 1.2783x; 1.2783x over previous
"""Trainium2 Bass kernel for nn_CrossAttention (B=8, L=1024, QD=1024, KVD=768, H=16).

Sharding: data-parallel over batch across the 8 NeuronCores (1 batch row each).
Per-core pipeline (bf16 matmuls, fp32 accumulation / residual / layernorm):
  A) per-128-row-block plain fp32 loads (HWDGE, full DMA rate), fp32->bf16
     conversion on compute engines (DVE for q/wq/k/wk, GpSimd for v/wv/wo —
     SWDGE converting DMA is ~5x slower than line rate, so it is avoided),
     then SBUF->SBUF xbar DMA transposes into block-major transposed layouts
     XTb[p, rb, cc, r] = X[rb*128+r, cc*128+p] (contiguous dst = fast path).
  B) projections: qhT/khT (per-partition bias; B1 evictions on DVE, B2 on
     ACT), vh natural with ones-augmented columns; key-padding mask folded
     into vh_aug rows so exp needs no mask bias. B3 interleaved into the
     first attention pair slots.
  C) attention per head pair: scoresT = khT.T @ qhT with the two heads of a
     pair issued back-to-back on distinct PE row groups (64-row tiling =>
     concurrent execution); exp split across engines: hh=0 on ACT (table
     exp), hh=1 on DVE via a one-instruction Schraudolph bit-trick straight
     to bf16 (the constant-factor part of its error cancels in softmax);
     attnV with [ones|vh] stationary giving psum rows 0:64 = replicated
     denominator and rows 64:128 = o; approx reciprocal + multiply on DVE.
  D) out-projection from oT stationary + rank-1 bias, fp32 residual + LN.
"""

import numpy as np

import concourse.bass as bass
import concourse.mybir as mybir
import concourse.tile as tile
from concourse import bacc
from concourse.bass_utils import run_bass_kernel_spmd

F32 = mybir.dt.float32
BF16 = mybir.dt.bfloat16
I16 = mybir.dt.int16
U8 = mybir.dt.uint8

B = 8
L = 1024
C = 1024      # QD
KV = 768      # KVD
H = 16
DH = 64
P = 128
LT = L // P          # 8 l-tiles
CT = C // P          # 8 contraction tiles (model dim)
KT = KV // P         # 6 contraction tiles (kv dim)
DT = C // P          # 8 d-tiles
NH = C // 512        # 2 free-dim halves (N=512 per PSUM bank)
SCALE = DH ** -0.5
EPS = 1e-5
LOG2E = 1.4426950408889634
# Schraudolph exp straight to bf16 bits: u16 = x*EXP_A + EXP_B, bitcast bf16.
EXP_A = 128.0 * LOG2E * SCALE
EXP_B = 128.0 * (127.0 - 0.043677) + 0.5

Exp = mybir.ActivationFunctionType.Exp
Sqrt = mybir.ActivationFunctionType.Sqrt
Identity = mybir.ActivationFunctionType.Identity
MULT = mybir.AluOpType.mult
ADD = mybir.AluOpType.add

TRACE = False
LAST_RESULT = None
_CACHE = {}


def _bcast_ap(handle, parts):
    apx = handle[:]
    return bass.AP(tensor=apx.tensor, offset=apx.offset,
                   ap=[[0, parts]] + [list(x) for x in apx.ap])


def build(apply_gb=False):
    nc = bacc.Bacc("TRN2", target_bir_lowering=False)

    q_in = nc.dram_tensor("q", [L, C], F32, kind="ExternalInput")
    k_in = nc.dram_tensor("k", [L, KV], F32, kind="ExternalInput")
    v_in = nc.dram_tensor("v", [L, KV], F32, kind="ExternalInput")
    m_in = nc.dram_tensor("key_padding_mask", [L], U8, kind="ExternalInput")
    wq_in = nc.dram_tensor("Wq", [C, C], F32, kind="ExternalInput")
    bq_in = nc.dram_tensor("bq", [C], F32, kind="ExternalInput")
    wk_in = nc.dram_tensor("Wk", [C, KV], F32, kind="ExternalInput")
    bk_in = nc.dram_tensor("bk", [C], F32, kind="ExternalInput")
    wv_in = nc.dram_tensor("Wv", [C, KV], F32, kind="ExternalInput")
    bv_in = nc.dram_tensor("bv", [C], F32, kind="ExternalInput")
    wo_in = nc.dram_tensor("Wo", [C, C], F32, kind="ExternalInput")
    bo_in = nc.dram_tensor("bo", [C], F32, kind="ExternalInput")
    gamma_in = nc.dram_tensor("gamma", [C], F32, kind="ExternalInput")
    beta_in = nc.dram_tensor("beta", [C], F32, kind="ExternalInput")
    y_out = nc.dram_tensor("y", [L, C], F32, kind="ExternalOutput")

    with tile.TileContext(nc) as tc:
        with (
            tc.tile_pool(name="cst", bufs=1) as cst,
            tc.tile_pool(name="persist", bufs=1) as persist,
            tc.tile_pool(name="xpO", bufs=1) as xpO,
        ):
            # ---------------- projection outputs (persist through attention)
            qhT = persist.tile([P, DT, L], BF16)          # d on partitions
            khT = persist.tile([P, DT, L], BF16)
            vh_aug = persist.tile([P, LT, H * P], BF16)   # per m-tile: 16x[64 ones | 64 vh]
            WoTb = xpO.tile([P, CT, DT, P], BF16)         # [p, cb, dc, r]
            oT = xpO.tile([P, DT, L], BF16)

            with (
                tc.tile_pool(name="xpV", bufs=1) as xpV,
                tc.tile_pool(name="dram", bufs=1, space="DRAM") as dram,
                tc.tile_pool(name="stg", bufs=6) as stg,
            ):
                WvTb = xpV.tile([P, DT, KT, P], BF16)
                vTb = xpV.tile([P, LT, KT, P], BF16)
                dram_bf = {}

                def stage_chunk(nm, hnd, rows, cols, ch, dstT):
                    # 256-row chunk: converting cast fp32->bf16 (SWDGE,
                    # sustains line rate when several are in flight), store
                    # bf16 to DRAM (HWDGE/ACT queue), then per-128-row-block
                    # DRAM->SBUF xbar transposes into the block-major world:
                    # dstT[p, b, cc, r] = X[b*128+r, cc*128+p].  DRAM-source
                    # transposes do not trigger the transpose|SBUF-SBUF DMA
                    # serialization guard the way SBUF-source ones do.
                    if nm not in dram_bf:
                        dram_bf[nm] = dram.tile([rows, cols], BF16,
                                                name=f"{nm}_bf", tag=f"{nm}_bf")
                    t = dram_bf[nm]
                    r0 = ch * 256
                    st = stg.tile([P, 2, cols], BF16, name=f"st_{nm}{ch}",
                                  tag="stg")
                    nc.gpsimd.dma_start(
                        st, hnd[r0:r0 + 256, :].rearrange("(b p) c -> p b c", p=P))
                    nc.scalar.dma_start(
                        t[r0:r0 + 256, :].rearrange("(b p) c -> p b c", p=P), st)
                    for bb in range(2):
                        b = ch * 2 + bb
                        nc.sync.dma_start(dstT[:, b, :, :],
                                          t[b * P:(b + 1) * P, :],
                                          transpose=True)

                with (
                    tc.tile_pool(name="xpQK", bufs=1) as xpQK,
                    tc.tile_pool(name="psum_b", bufs=3, space="PSUM") as psum_b,
                ):
                    WqTb = xpQK.tile([P, DT, CT, P], BF16)
                    qTb = xpQK.tile([P, LT, CT, P], BF16)
                    WkTb = xpQK.tile([P, DT, KT, P], BF16)
                    kTb = xpQK.tile([P, LT, KT, P], BF16)

                    # tiny consts first (they gate vh masking / evictions)
                    bq_sb = cst.tile([P, DT], F32)
                    nc.gpsimd.dma_start(bq_sb, bq_in[:].rearrange("(t p) -> p t", p=P))
                    bk_sb = cst.tile([P, DT], F32)
                    nc.gpsimd.dma_start(bk_sb, bk_in[:].rearrange("(t p) -> p t", p=P))
                    mask_u8 = cst.tile([P, LT], U8)
                    nc.gpsimd.dma_start(mask_u8, m_in[:].rearrange("(t p) -> p t", p=P))
                    mask01 = cst.tile([P, LT], F32)
                    nc.vector.tensor_copy(mask01, mask_u8)
                    ones_row = cst.tile([1, P], BF16)
                    nc.vector.memset(ones_row, 1.0)
                    eps_sb = cst.tile([P, 1], F32)
                    nc.vector.memset(eps_sb, EPS)
                    bv_bf = cst.tile([1, C], BF16)
                    nc.gpsimd.dma_start(bv_bf, bv_in[:].rearrange("(a c) -> a c", a=1))
                    bo_bf = cst.tile([1, C], BF16)
                    nc.gpsimd.dma_start(bo_bf, bo_in[:].rearrange("(a c) -> a c", a=1))
                    if apply_gb:
                        gamma_b = cst.tile([P, C], F32)
                        nc.gpsimd.dma_start(gamma_b, _bcast_ap(gamma_in, P))
                        beta_b = cst.tile([P, C], F32)
                        nc.gpsimd.dma_start(beta_b, _bcast_ap(beta_in, P))
                    else:
                        gamma_b = beta_b = None

                    # vh_aug: ones columns masked by key_padding (folds the
                    # mask out of exp entirely: padded m-rows contribute 0 to
                    # both numerator and denominator).
                    nc.gpsimd.memset(vh_aug[:], 1.0)
                    for mt in range(LT):
                        ones_cols = vh_aug[:, mt, :].rearrange(
                            "p (h x) -> p h x", x=P)[:, :, 0:DH]
                        nc.vector.tensor_scalar_mul(ones_cols, ones_cols,
                                                    mask01[:, mt:mt + 1])

                    # ---- stage q/wq (gates B1), then k/wk (B2 runs behind),
                    # then v/wv (B3) and wo (out-projection), all pipelined
                    # on the gpsimd(cast)/scalar(store)/sync(transpose) queues
                    for ch in range(2):
                        stage_chunk("q", q_in, L, C, ch, qTb)
                    stage_chunk("wq", wq_in, C, C, 0, WqTb)
                    for ch in range(2, 4):
                        stage_chunk("q", q_in, L, C, ch, qTb)
                    for ch in range(1, 4):
                        stage_chunk("wq", wq_in, C, C, ch, WqTb)
                    for ch in range(2):
                        stage_chunk("k", k_in, L, KV, ch, kTb)
                    stage_chunk("wk", wk_in, C, KV, 0, WkTb)
                    for ch in range(2, 4):
                        stage_chunk("k", k_in, L, KV, ch, kTb)
                    for ch in range(1, 4):
                        stage_chunk("wk", wk_in, C, KV, ch, WkTb)
                    stage_chunk("v", v_in, L, KV, 0, vTb)
                    for ch in range(2):
                        stage_chunk("wv", wv_in, C, KV, ch, WvTb)
                    stage_chunk("v", v_in, L, KV, 1, vTb)
                    for ch in range(2, 4):
                        stage_chunk("wv", wv_in, C, KV, ch, WvTb)
                    for ch in range(2, 4):
                        stage_chunk("v", v_in, L, KV, ch, vTb)
                    for ch in range(4):
                        stage_chunk("wo", wo_in, C, C, ch, WoTb)

                    # ---- B1: qhT[d, l]   (evictions on DVE)
                    for dt in range(DT):
                        for lh in range(NH):
                            ps = psum_b.tile([P, 512], F32, tag="ps")
                            for ct in range(CT):
                                nc.tensor.matmul(
                                    ps, WqTb[:, dt, ct, :],
                                    qTb[:, lh * 4:(lh + 1) * 4, ct, :],
                                    start=(ct == 0), stop=(ct == CT - 1))
                            nc.vector.tensor_scalar_add(
                                qhT[:, dt, lh * 512:(lh + 1) * 512], ps,
                                bq_sb[:, dt:dt + 1])

                    # ---- B2: khT[d, l]   (evictions on ACT)
                    for dt in range(DT):
                        for lh in range(NH):
                            ps = psum_b.tile([P, 512], F32, tag="ps")
                            for ct in range(KT):
                                nc.tensor.matmul(
                                    ps, WkTb[:, dt, ct, :],
                                    kTb[:, lh * 4:(lh + 1) * 4, ct, :],
                                    start=(ct == 0), stop=(ct == KT - 1))
                            nc.scalar.activation(
                                khT[:, dt, lh * 512:(lh + 1) * 512], ps,
                                Identity, bias=bk_sb[:, dt:dt + 1])

                # ---------------- attention, with B3 (vh projection)
                # interleaved into the first two pair slots
                with (
                    tc.tile_pool(name="ptp", bufs=25) as ptp,
                    tc.tile_pool(name="recp", bufs=2) as recp,
                    tc.tile_pool(name="psum_sc", bufs=2, space="PSUM") as psum_sc,
                    tc.tile_pool(name="psum_av", bufs=1, space="PSUM") as psum_av,
                    tc.tile_pool(name="psum_b3", bufs=2, space="PSUM") as psum_b3,
                ):
                    pts = {}

                    def scores_exp(pair):
                        for mt in range(LT):
                            sc = []
                            for hh in range(2):
                                s = psum_sc.tile([P, L], F32,
                                                 name=f"sc{pair}_{mt}_{hh}", tag="sc")
                                sc.append(s)
                            # interleave the two heads' matmuls: distinct
                            # 64-row PE groups execute them concurrently
                            for lh in range(NH):
                                for hh in range(2):
                                    p0 = hh * DH
                                    nc.tensor.matmul(
                                        sc[hh][:, lh * 512:(lh + 1) * 512],
                                        khT[p0:p0 + DH, pair, mt * P:(mt + 1) * P],
                                        qhT[p0:p0 + DH, pair, lh * 512:(lh + 1) * 512],
                                        start=True, stop=True)
                            for hh in range(2):
                                pt = ptp.tile([P, L], BF16,
                                              name=f"pt{pair}_{mt}_{hh}", tag="pt")
                                pts[(pair, mt, hh)] = pt
                                if hh == 0:
                                    nc.scalar.activation(pt, sc[hh], Exp,
                                                         scale=SCALE)
                                else:
                                    nc.vector.tensor_scalar(
                                        pt[:].bitcast(I16), sc[hh],
                                        EXP_A, EXP_B, MULT, ADD)

                    def b3_chunk(mts):
                        for mt in mts:
                            for dh2 in range(NH):
                                ps = psum_b3.tile([P, 512], F32, tag="ps3")
                                for ct in range(KT):
                                    nc.tensor.matmul(
                                        ps, vTb[:, mt, ct, :],
                                        WvTb[:, dh2 * 4:(dh2 + 1) * 4, ct, :],
                                        start=(ct == 0), stop=False)
                                nc.tensor.matmul(
                                    ps, ones_row[0:1, :],
                                    bv_bf[0:1, dh2 * 512:(dh2 + 1) * 512],
                                    start=False, stop=True)
                                dst = vh_aug[:, mt, :].rearrange(
                                    "p (h x) -> p h x", x=P)
                                dst = dst[:, dh2 * 8:(dh2 + 1) * 8, DH:P]
                                # eviction also applies the key-padding mask
                                nc.vector.tensor_scalar_mul(
                                    dst, ps[:].rearrange("p (h d) -> p h d", d=DH),
                                    mask01[:, mt:mt + 1])

                    def attnv(pair):
                        for hh in range(2):
                            h = 2 * pair + hh
                            av = psum_av.tile([P, L], F32,
                                              name=f"av{pair}_{hh}", tag="av")
                            # mt-outer: both l-halves reuse each vh stationary load
                            for mt in range(LT):
                                for lh in range(NH):
                                    nc.tensor.matmul(
                                        av[:, lh * 512:(lh + 1) * 512],
                                        vh_aug[:, mt, h * P:(h + 1) * P],
                                        pts[(pair, mt, hh)][:, lh * 512:(lh + 1) * 512],
                                        start=(mt == 0), stop=(mt == LT - 1))
                            rec = recp.tile([P, L], F32,
                                            name=f"rec{pair}_{hh}", tag="rec")
                            nc.vector.reciprocal_approx_fast(rec[0:DH, :],
                                                             av[0:DH, :])
                            nc.vector.tensor_mul(
                                oT[hh * DH:(hh + 1) * DH, pair, :],
                                av[DH:P, :], rec[0:DH, :])
                            for mt in range(LT):
                                del pts[(pair, mt, hh)]

                    scores_exp(0)
                    b3_chunk(range(0, 4))
                    scores_exp(1)
                    b3_chunk(range(4, 8))
                    attnv(0)
                    for pair in range(2, H // 2):
                        scores_exp(pair)
                        attnv(pair - 1)
                    attnv(H // 2 - 1)

            # ---------------- out-projection + residual + layernorm
            with (
                tc.tile_pool(name="dwork", bufs=3) as dwork,
                tc.tile_pool(name="dsmall", bufs=8) as dsmall,
                tc.tile_pool(name="psum_y", bufs=3, space="PSUM") as psum_y,
            ):
                qrs = []
                for lt in range(LT):
                    qr = dwork.tile([P, C], F32, name=f"qr{lt}", tag="qr",
                                    bufs=8)
                    nc.sync.dma_start(qr, q_in[lt * P:(lt + 1) * P, :])
                    qrs.append(qr)
                for lt in range(LT):
                    yp = psum_y.tile([P, C], F32, tag="yp")
                    # dt-outer: the oT stationary serves both column halves
                    for dt in range(DT):
                        for ch in range(NH):
                            nc.tensor.matmul(
                                yp[:, ch * 512:(ch + 1) * 512],
                                oT[:, dt, lt * P:(lt + 1) * P],
                                WoTb[:, ch * 4:(ch + 1) * 4, dt, :],
                                start=(dt == 0), stop=False)
                    for ch in range(NH):
                        nc.tensor.matmul(
                            yp[:, ch * 512:(ch + 1) * 512],
                            ones_row[0:1, :],
                            bo_bf[0:1, ch * 512:(ch + 1) * 512],
                            start=False, stop=True)
                    ysb = dwork.tile([P, C], F32, tag="ysb")
                    nc.vector.tensor_add(ysb, yp, qrs[lt])
                    st = dsmall.tile([P, 2, 6], F32, tag="st")
                    nc.vector.bn_stats(st[:, 0, :], ysb[:, 0:512])
                    nc.vector.bn_stats(st[:, 1, :], ysb[:, 512:1024])
                    mv = dsmall.tile([P, 2], F32, tag="mv")
                    nc.vector.bn_aggr(mv, st)
                    rstd = dsmall.tile([P, 1], F32, tag="rstd")
                    nc.scalar.activation(rstd, mv[:, 1:2], Sqrt,
                                         bias=eps_sb[:, 0:1])
                    nc.vector.reciprocal(rstd, rstd)
                    nmr = dsmall.tile([P, 1], F32, tag="nmr")
                    nc.vector.tensor_mul(nmr, mv[:, 0:1], rstd)
                    nc.vector.tensor_scalar_mul(nmr, nmr, -1.0)
                    yn = dwork.tile([P, C], F32, tag="yn")
                    nc.scalar.activation(yn, ysb, Identity, bias=nmr[:, 0:1],
                                         scale=rstd[:, 0:1])
                    if apply_gb:
                        nc.vector.tensor_mul(yn, yn, gamma_b)
                        nc.gpsimd.tensor_add(yn, yn, beta_b)
                    nc.sync.dma_start(y_out[lt * P:(lt + 1) * P, :], yn)

    nc.compile()
    return nc


def _get_nc(apply_gb):
    key = ("nc", apply_gb)
    if key not in _CACHE:
        _CACHE[key] = build(apply_gb)
    return _CACHE[key]


def kernel(**inputs) -> np.ndarray:
    global LAST_RESULT
    gamma = np.asarray(inputs["gamma"], dtype=np.float32)
    beta = np.asarray(inputs["beta"], dtype=np.float32)
    apply_gb = not (np.all(gamma == 1.0) and np.all(beta == 0.0))
    nc = _get_nc(apply_gb)
    q = np.ascontiguousarray(np.asarray(inputs["q"], dtype=np.float32))
    k = np.ascontiguousarray(np.asarray(inputs["k"], dtype=np.float32))
    v = np.ascontiguousarray(np.asarray(inputs["v"], dtype=np.float32))
    mask = np.ascontiguousarray(np.asarray(inputs["key_padding_mask"]).astype(np.uint8))
    shared = {
        name: np.ascontiguousarray(np.asarray(inputs[name], dtype=np.float32))
        for name in ("Wq", "bq", "Wk", "bk", "Wv", "bv", "Wo", "bo", "gamma", "beta")
    }
    in_maps = []
    for b in range(B):
        m = {"q": q[b], "k": k[b], "v": v[b], "key_padding_mask": mask[b]}
        m.update(shared)
        in_maps.append(m)
    LAST_RESULT = run_bass_kernel_spmd(nc, in_maps, core_ids=list(range(B)), trace=TRACE)
    return np.stack([r["y"] for r in LAST_RESULT.results], axis=0)
